# revision 104
# baseline (speedup 1.0000x reference)
"""DigitCaps dynamic-routing kernel for 8 Trainium2 NeuronCores.

Strategy (batch-sharded, fully local per core, no collectives):
  B=256 -> 8 cores x 32 batch rows; u_hat resident in SBUF as bf16 in layout
      u_hat[p = 32*(i%4) + b, free = (i//4)*160 + k*10 + o]

  Creation: ONE full-array matmul per j-round (i = 4j+r): lhsT is a
  host-packed block-diagonal x image [32=(r,d), 128=(r,b)] so all 128 out
  partitions fill per instruction (4x fewer PE rows than per-quadrant
  tiling). PSUM drains to bf16 split DVE/ACT. s1 (uniform-c iteration 1)
  uses a dense (i,d)=128 contraction, sandwiched mid-creation so squash(v1)
  and the first cdiag build hide under the remaining drains.

  Agreement passes run on the PE as diagonal matmuls: cdiag[(ko)] =
  mask * v[p,ko] (built 2x-mode via pair duplication, per-o stripes,
  DVE+Pool) then out[128, 96-j] += diag(v_ko) @ u[:, (j, ko)], accumulating
  over k into rotating 1-bank PSUM pieces. Each piece's 16 accumulating
  matmuls are consecutive (interleaved start/stop groups in one bank
  corrupt sibling slots). Softmax state is eL = exp(logits) in SBUF bf16,
  o-major; iteration 3 uses the multiplicative update eL *= exp(a_delta)
  so no logits tensor is ever resident. Z = per-j reduce + reciprocal;
  c16 in one strided multiply per 96-j group.

  s-pass: t = c (.) u_hat on DVE (bf16 2x; Pool takes 2 of 9 blocks), then
  the PE folds sum_j sum_{i%4} via ones-block-diagonal accumulating
  matmuls into s PSUM [32,160]. The three 96-j groups are software-
  pipelined: PE agreement of group N+1 is emitted before the folds of
  group N so the PE never stalls on DVE.

  squash on [32,*] tiles (DVE + one ACT sqrt); v is replicated x4 via a
  onesT matmul and consumed straight from PSUM by the next cdiag build.
"""

import numpy as np

B, NI, DI, NO, K = 256, 1152, 8, 10, 16
NC = 8
BL = B // NC            # 32 batch rows per core
NJ = NI // 4            # 288 j-rounds (i = 4*j + r)
OK = NO * K             # 160, stored in (k, o) order: idx = k*10 + o
NQ = NI // 16           # 72 dense chunks (i = 16*q + t)
EPS = 1e-9

# creation PSUM chunking: 6 j-rounds per psum tile, 3 j per 512-col bank slot
JC = 6
# DVE block size (j per block) for routing passes
JB = 32

_CACHE = {}


def _pack_inputs(x, W, bias):
    """Host-side packing into per-core DMA images (all plain contiguous)."""
    import ml_dtypes
    bf16 = ml_dtypes.bfloat16

    # xblk (creation lhsT, block-diagonal over i_sub): [32, NJ*128]
    #   xblk[8*r' + d, j*128 + r*32 + b] = x[b0+b, 4*j+r, d] * (r'==r)
    # one [32,128] weight tile per j -> single full-array matmul per j.
    xr = x.reshape(B, NJ, 4, DI)                       # [b, j, r, d]
    xt_all = np.ascontiguousarray(
        xr.transpose(2, 3, 1, 0)).astype(bf16)         # [r, d, j, b_all]

    # dense xtD (s1 lhsT): [128, NQ*32]: xtD[16*?] rows = (t, d) = 128
    xd = x.reshape(B, NQ, 16, DI)                      # [b, q, t, d]
    xtD_all = np.ascontiguousarray(
        xd.transpose(2, 3, 1, 0)).astype(bf16)         # [t, d, q, b_all]

    # wp (creation rhs stream): dense rows [32, NJ*OK]
    #   wp[8*r + d, j*160 + k*10 + o] = W[4*j+r, o, k, d]
    wr = W.reshape(NJ, 4, NO, K, DI)                   # [j, r, o, k, d]
    wp = np.ascontiguousarray(
        wr.transpose(1, 4, 0, 3, 2).reshape(32, NJ * OK)).astype(bf16)

    # wd (s1 rhs stream, dense): [128, NQ*OK]
    #   wd[8*t + d, q*160 + k*10 + o] = W[16*q+t, o, k, d]
    wq = W.reshape(NQ, 16, NO, K, DI)                  # [q, t, o, k, d]
    wd = np.ascontiguousarray(
        wq.transpose(1, 4, 0, 3, 2).reshape(128, NQ * OK)).astype(bf16)

    # ones block-diag for folding 4 partition groups: [128, 32]
    ones_bd = np.zeros((128, BL), dtype=np.float32)
    for c in range(4):
        ones_bd[np.arange(BL) + 32 * c, np.arange(BL)] = 1.0
    ones_bd = ones_bd.astype(bf16)

    # onesT for partition replication via PE: [32, 128]
    onesT = np.zeros((BL, 128), dtype=np.float32)
    for c in range(4):
        onesT[np.arange(BL), np.arange(BL) + 32 * c] = 1.0
    onesT = onesT.astype(bf16)

    # bias replicated [32, 160] f32 in (k, o) order
    biasr = np.ascontiguousarray(
        np.broadcast_to(bias.T.reshape(1, OK), (BL, OK))).astype(np.float32)

    # identity mask for per-partition diag matmuls: [128, 128]
    mask = np.eye(128, dtype=np.float32).astype(bf16)
    xzero = np.zeros((32, 36 * 128), dtype=bf16)

    per_core = []
    for cid in range(NC):
        b0 = cid * BL
        xs = xt_all[:, :, :, b0:b0 + BL]               # [r, d, j, b]
        xblk = np.zeros((4, DI, NJ, 4, BL), dtype=bf16)
        for s in range(4):
            xblk[s, :, :, s, :] = xs[s]
        xblk = np.ascontiguousarray(xblk.reshape(32, NJ * 128))
        xtD = np.ascontiguousarray(
            xtD_all[:, :, :, b0:b0 + BL].reshape(128, NQ * BL))
        per_core.append({
            "xblk": xblk, "xtd": xtD, "wp": wp, "wd": wd,
            "ones_bd": ones_bd, "onesT": onesT, "biasr": biasr,
            "mask": mask,
        })
    return per_core


def _build_bass(debug=False, upto=99):
    import concourse.bass as bassm
    import concourse.bacc as bacc
    import concourse.mybir as mybir
    from concourse.tile import TileContext

    dt = mybir.dt
    ALU = mybir.AluOpType
    ACTF = mybir.ActivationFunctionType
    AX = mybir.AxisListType

    nc = bacc.Bacc()

    xblk_d = nc.dram_tensor("xblk", [32, NJ * 128], dt.bfloat16, kind="ExternalInput")
    xtD_d = nc.dram_tensor("xtd", [128, NQ * BL], dt.bfloat16, kind="ExternalInput")
    wp_d = nc.dram_tensor("wp", [32, NJ * OK], dt.bfloat16, kind="ExternalInput")
    wd_d = nc.dram_tensor("wd", [128, NQ * OK], dt.bfloat16, kind="ExternalInput")
    ones_d = nc.dram_tensor("ones_bd", [128, BL], dt.bfloat16, kind="ExternalInput")
    onesT_d = nc.dram_tensor("onesT", [BL, 128], dt.bfloat16, kind="ExternalInput")
    biasr_d = nc.dram_tensor("biasr", [BL, OK], dt.float32, kind="ExternalInput")
    mask_d = nc.dram_tensor("mask", [128, 128], dt.bfloat16, kind="ExternalInput")
    out_d = nc.dram_tensor("out_v", [BL, OK], dt.float32, kind="ExternalOutput")
    if debug:
        dbg_u = nc.dram_tensor("dbg_u", [128, NJ * OK], dt.bfloat16, kind="ExternalOutput")
        dbg_eL = nc.dram_tensor("dbg_eL", [128, NO * NJ], dt.bfloat16, kind="ExternalOutput")
        dbg_c = nc.dram_tensor("dbg_c", [128, NJ * NO], dt.bfloat16, kind="ExternalOutput")
        dbg_v1 = nc.dram_tensor("dbg_v1", [128, OK], dt.bfloat16, kind="ExternalOutput")

    with TileContext(nc) as tc:
        with (
            tc.tile_pool(name="const", bufs=1) as const,
            tc.tile_pool(name="big", bufs=1) as big,
            tc.tile_pool(name="small", bufs=1) as small,
            tc.tile_pool(name="pss", bufs=1, space="PSUM") as pss,
            tc.tile_pool(name="psA", bufs=2, space="PSUM") as psA,
        ):
            # ---- resident tiles ----
            u_hat = big.tile([128, NJ * OK], dt.bfloat16)     # 90KB/part
            cdiag = big.tile([128, OK * 128], dt.bfloat16)    # 40KB/part
            ones_sb = const.tile([128, BL], dt.bfloat16)
            onesT_sb = const.tile([BL, 128], dt.bfloat16)
            biasr_sb = const.tile([BL, OK], dt.float32)
            mask_sb = const.tile([128, 128], dt.bfloat16)
            vrep2 = const.tile([128, 2 * OK], dt.bfloat16)    # (ko, 2) pairs
            vrep_ps = [None]  # squash's replicate-PSUM, read by cdiag_build

            s1_ps = pss.tile([BL, OK], dt.float32, tag="sps")

            CD = cdiag.rearrange("p (t f) -> p t f", f=128)    # [p,160,128]
            UJ = u_hat.rearrange("p (j f) -> p j f", f=OK)     # [p,288,160]

            def cdiag_build():
                """cdiag[p,ko,f] = mask[p,f] * v[p,ko]; DVE+Pool.

                Reads v replicated straight from the squash's PSUM tile
                (vrep_ps[0]). The pair duplication vrep2[p,(ko,2)] makes
                the DVE ops' innermost dim a packed stride-1 pair -> 2x
                DVE mode."""
                v2v = vrep2.rearrange("p (t two) -> p t two", two=2)
                vin = vrep_ps[0].rearrange("p (t a) -> p t a", a=1)
                vi2, _ = bassm.broadcast_tensor_aps(vin, v2v)
                nc.vector.tensor_copy(v2v, vi2)
                mk4 = mask_sb.rearrange(
                    "p (a h two) -> p a h two", a=1, two=2)   # [p,1,64,2]
                vv5 = vrep2.rearrange(
                    "p (k o a two) -> p o k a two",
                    k=K, o=NO, a=1, two=2)                    # [p,10,16,1,2]
                CD5 = cdiag.rearrange(
                    "p (k o h two) -> p o k h two",
                    k=K, o=NO, h=64, two=2)                   # [p,10,16,64,2]
                # per-o stripes (pair index = k*10 + o: stride over k) so
                # the agreement pieces (o-major) can start as stripes land;
                # o=0..7 on DVE, o=8,9 on Pool
                for o in range(NO):
                    eng = nc.vector if o < 8 else nc.gpsimd
                    m2, v2 = bassm.broadcast_tensor_aps(mk4, vv5[:, o])
                    eng.tensor_tensor(CD5[:, o], m2, v2, ALU.mult)

            def squash_to_vrep(s_ps, store_out=False):
                """v = squash(s) from PSUM [32,160] (+bias);
                replicate to vrep [128,160] bf16 (or DMA out if final)."""
                s_sb = small.tile([BL, OK], dt.float32, tag="s_sb")
                nc.vector.scalar_tensor_tensor(
                    s_sb[:, :], s_ps[:, :], 0.1 if store_out is None else 1.0,
                    biasr_sb[:, :], ALU.mult, ALU.add)
                sq = small.tile([BL, OK], dt.float32, tag="sq")
                nc.vector.tensor_mul(sq[:, :], s_sb[:, :], s_sb[:, :])
                n2 = small.tile([BL, NO], dt.float32, tag="n2")
                nc.vector.tensor_reduce(
                    n2[:, :],
                    sq.rearrange("p (k o) -> p o k", o=NO),
                    AX.X, ALU.add)
                n2e = small.tile([BL, NO], dt.float32, tag="n2e")
                nc.vector.tensor_scalar_add(n2e[:, :], n2[:, :], EPS)
                sr = small.tile([BL, NO], dt.float32, tag="sr")
                nc.scalar.activation(sr[:, :], n2e[:, :], ACTF.Sqrt)
                den = small.tile([BL, NO], dt.float32, tag="den")
                nc.vector.scalar_tensor_tensor(
                    den[:, :], n2[:, :], 1.0, sr[:, :], ALU.add, ALU.mult)
                rec = small.tile([BL, NO], dt.float32, tag="rec")
                nc.vector.reciprocal(rec[:, :], den[:, :])
                g = small.tile([BL, NO], dt.float32, tag="g")
                nc.vector.tensor_mul(g[:, :], n2[:, :], rec[:, :])
                sv = s_sb.rearrange("p (k o) -> p k o", o=NO)
                gv = g.rearrange("p (a o) -> p a o", a=1)
                sv2, gv2 = bassm.broadcast_tensor_aps(sv, gv)
                if store_out:
                    v_sb = small.tile([BL, OK], dt.float32, tag="v_sb")
                    nc.vector.tensor_tensor(
                        v_sb.rearrange("p (k o) -> p k o", o=NO), sv2, gv2,
                        ALU.mult)
                    nc.sync.dma_start(out_d[:, :], v_sb[:, :])
                    return
                v16 = small.tile([BL, OK], dt.bfloat16, tag="v16")
                nc.vector.tensor_tensor(
                    v16.rearrange("p (k o) -> p k o", o=NO), sv2, gv2,
                    ALU.mult)
                vr_ps = pss.tile([128, OK], dt.float32, tag="vr_ps")
                nc.tensor.matmul(
                    vr_ps[:, :], onesT_sb[:, :], v16[:, :],
                    start=True, stop=True)
                vrep_ps[0] = vr_ps

            # =========== creation phase (scoped pools) ===========
            with (
                tc.tile_pool(name="xw", bufs=2) as xw,
                tc.tile_pool(name="crp", bufs=1) as crp,
                tc.tile_pool(name="ps", bufs=2, space="PSUM") as psp,
            ):
                xtD_sb = crp.tile([128, NQ * BL], dt.bfloat16)
                wd_sb = crp.tile([128, NQ * OK], dt.bfloat16)
                JD = 36

                def s1_and_v1():
                    # s1: dense (i,d)=128 contraction; v1 = squash(.1*s1+b)
                    for q in range(NQ):
                        nc.tensor.matmul(
                            s1_ps[:, :],
                            xtD_sb[:, q * BL:(q + 1) * BL],
                            wd_sb[:, q * OK:(q + 1) * OK],
                            start=(q == 0), stop=(q == NQ - 1),
                        )
                    squash_to_vrep(s1_ps, store_out=None)
                    cdiag_build()

                # u_hat: one [32x128] matmul per j (block-diag xb lhsT,
                # expanded on-chip from the dense xt stream)
                for ci, jd in enumerate(range(0, NJ, JD)):
                    xb = xw.tile([32, JD * 128], dt.bfloat16, tag="xb")
                    wp_ch = xw.tile([32, JD * OK], dt.bfloat16, tag="wp")
                    nc.sync.dma_start(
                        xb[:, :], xblk_d[:, jd * 128:(jd + JD) * 128])
                    nc.gpsimd.dma_start(
                        wp_ch[:, :], wp_d[:, jd * OK:(jd + JD) * OK])
                    if jd == JD:
                        # consts + s1 dense inputs: issue after the first
                        # stream chunks so they don't delay creation start
                        nc.sync.dma_start(ones_sb[:, :], ones_d[:, :])
                        nc.sync.dma_start(onesT_sb[:, :], onesT_d[:, :])
                        nc.sync.dma_start(biasr_sb[:, :], biasr_d[:, :])
                        nc.sync.dma_start(mask_sb[:, :], mask_d[:, :])
                        nc.sync.dma_start(xtD_sb[:, :], xtD_d[:, :])
                        nc.gpsimd.dma_start(wd_sb[:, :], wd_d[:, :])
                    if jd == 5 * JD:
                        # sandwich s1 + v1 + cdiag1 into the engine queues
                        # so they complete while late drains still run
                        s1_and_v1()
                    for jc in range(0, JD, JC):
                        cps = psp.tile([128, 1024], dt.float32, tag="cps")
                        for jj in range(JC):
                            off = (jj // 3) * 512 + (jj % 3) * OK
                            nc.tensor.matmul(
                                cps[:, off:off + OK],
                                xb[:, (jc + jj) * 128:(jc + jj + 1) * 128],
                                wp_ch[:, (jc + jj) * OK:(jc + jj + 1) * OK],
                                start=True, stop=True,
                            )
                        # drain 6 j (strided: 2 banks x 480 cols) -> bf16
                        src = cps.rearrange("p (a x) -> p a x", a=2)[:, :, 0:3 * OK]
                        dst = u_hat[:, (jd + jc) * OK:(jd + jc + JC) * OK].rearrange(
                            "p (a x) -> p a x", a=2)
                        if ((jd + jc) // JC) % 3 == 0:
                            nc.vector.tensor_copy(dst, src)
                        else:
                            nc.scalar.copy(dst, src)

            # =========== iteration helpers ===========
            # =========== routing phase ===========
            with tc.tile_pool(name="tmp", bufs=3) as tmpp:
                eL = tmpp.tile([128, NO * NJ], dt.bfloat16, bufs=1, tag="eL")
                z5 = tmpp.tile([128, 5 * NJ], dt.bfloat16, bufs=1, tag="z5")
                zrec = tmpp.tile([128, NJ], dt.bfloat16, bufs=1, tag="zrec")
                c16 = tmpp.tile([128, NJ * NO], dt.bfloat16, bufs=1, tag="c16")
                ELo = eL.rearrange("p (o j) -> p o j", j=NJ)   # [p,10,288]
                Z5 = z5.rearrange("p (o j) -> p o j", j=NJ)    # [p,5,288]
                Cj = c16.rearrange("p (j o) -> p j o", o=NO)   # [p,288,10]

                def routing_iter(update, build_diag):
                    """One fused routing iteration: agreement -> softmax ->
                    s-pass, pipelined across the three jp-groups of 96 j so
                    PE agreement of group N+1 overlaps DVE softmax/s-mult of
                    group N. Returns the s PSUM [32,160].

                    Agreement accumulates (over k) into rotating 1-bank PSUM
                    pieces [128, 5x96]; softmax state is eL (SBUF bf16).
                    update=False: eL = exp(a); update=True (later iters):
                    eL *= exp(a_delta)  -- multiplicative softmax update, no
                    resident logits needed.

                    Pool computes the slow s-mult t-blocks 0,1 (their c16
                    group is ready first); their folds close the chain so
                    PE never waits on Pool."""
                    if build_diag:
                        cdiag_build()
                    s_ps = pss.tile([BL, OK], dt.float32, tag="sps")
                    # fold order: pool blocks 0,1 last; first fold = block 2
                    fold_sched = {0: [2], 1: [3, 0, 4, 5], 2: [6, 1, 7, 8]}
                    t_tiles = {}
                    started = [False]

                    def emit_agr(jp):
                        # k outer: the first matmuls need only the first
                        # cdiag range, so agreement starts ~1us into the
                        # cdiag build instead of after it
                        aps = []
                        for oh in range(2):
                            ap = psA.tile([128, 480], dt.float32, tag="ap",
                                          name=f"ap{jp}{oh}")
                            aps.append(ap)
                        # o outer, k inner: each piece's 16 accumulating
                        # matmuls are CONSECUTIVE -- interleaved start/stop
                        # groups within one PSUM bank corrupt earlier slots
                        for o in range(NO):
                            for k in range(K):
                                ko = k * NO + o
                                slot = o % 5
                                nc.tensor.matmul(
                                    aps[o // 5][:, slot * 96:(slot + 1) * 96],
                                    CD[:, ko, :],
                                    UJ[:, jp * 96:(jp + 1) * 96, ko],
                                    start=(k == 0),
                                    stop=(k == K - 1),
                                )
                        return aps

                    def emit_softmax(jp, aps):
                        js = slice(jp * 96, (jp + 1) * 96)
                        for oh in range(2):
                            src = aps[oh].rearrange("p (s j) -> p s j", j=96)
                            dstv = ELo[:, oh * 5:(oh + 1) * 5, js]
                            if not update:
                                nc.scalar.activation(dstv, src, ACTF.Exp)
                            else:
                                ex = tmpp.tile([128, 480], dt.bfloat16,
                                               tag="ex", name="ex", bufs=2)
                                exv = ex.rearrange("p (s j) -> p s j", j=96)
                                nc.scalar.activation(exv, src, ACTF.Exp)
                                nc.vector.tensor_tensor(
                                    dstv, dstv, exv, ALU.mult)
                        eJO = eL.rearrange("p (o j) -> p j o", o=NO)
                        with nc.allow_low_precision(
                                reason="Z/c in bf16; error averages over i"):
                            nc.vector.tensor_reduce(
                                z5[:, js], eJO[:, js, :], AX.X, ALU.add)
                            nc.vector.reciprocal(zrec[:, js], z5[:, js])
                        zb = zrec.rearrange("p (j a) -> p j a", a=1)
                        e2, z2 = bassm.broadcast_tensor_aps(
                            eJO[:, js, :], zb[:, js, :])
                        nc.vector.tensor_tensor(
                            Cj[:, js, :], e2, z2, ALU.mult)

                    def emit_smult(jp):
                        # blocks 0,1 on Pool; the rest on DVE
                        for bi in (3 * jp, 3 * jp + 1, 3 * jp + 2):
                            jb = bi * JB
                            pool_blk = bi <= 1
                            tag = "tp" if pool_blk else "t"
                            t = tmpp.tile([128, JB * OK], dt.bfloat16,
                                          tag=tag, name=f"t_{tag}",
                                          bufs=2 if pool_blk else 3)
                            t_tiles[bi] = t
                            tv = t.rearrange("p (j k o) -> p j k o",
                                             j=JB, k=K)
                            uv = u_hat[:, jb * OK:(jb + JB) * OK].rearrange(
                                "p (j k o) -> p j k o", j=JB, k=K)
                            cv = Cj[:, jb:jb + JB, :].rearrange(
                                "p j (a o) -> p j a o", a=1)
                            uv2, cv2 = bassm.broadcast_tensor_aps(uv, cv)
                            eng = nc.gpsimd if pool_blk else nc.vector
                            eng.tensor_tensor(tv, uv2, cv2, ALU.mult)

                    def emit_folds(jp):
                        for bi in fold_sched[jp]:
                            t = t_tiles[bi]
                            for jj in range(JB):
                                nc.tensor.matmul(
                                    s_ps[:, :], ones_sb[:, :],
                                    t[:, jj * OK:(jj + 1) * OK],
                                    start=not started[0],
                                    stop=(bi == 8 and jj == JB - 1))
                                started[0] = True

                    # software-pipelined: folds of group N trail the
                    # agreement matmuls of group N+1 on the PE queue
                    ap0 = emit_agr(0)
                    emit_softmax(0, ap0)
                    emit_smult(0)
                    ap1 = emit_agr(1)
                    emit_folds(0)
                    emit_softmax(1, ap1)
                    emit_smult(1)
                    ap2 = emit_agr(2)
                    emit_folds(1)
                    emit_softmax(2, ap2)
                    emit_smult(2)
                    emit_folds(2)
                    return s_ps

                # =========== routing ===========
                # (iter 1 = s1+squash1+cdiag1 was sandwiched mid-creation)
                if debug:
                    nc.sync.dma_start(dbg_u[:, :], u_hat[:, :])
                    nc.sync.dma_start(
                        dbg_v1[:, :],
                        vrep2.rearrange("p (t two) -> p t two", two=2)[:, :, 0])
                # iter 2 (cdiag1 already built mid-creation)
                s2 = routing_iter(update=False, build_diag=False)
                if debug:
                    nc.sync.dma_start(dbg_eL[:, :], eL[:, :])
                    nc.sync.dma_start(dbg_c[:, :], c16[:, :])
                squash_to_vrep(s2)                  # v2 -> vrep
                # iter 3: eL *= exp(a2)
                s3 = routing_iter(update=True, build_diag=True)
                squash_to_vrep(s3, store_out=True)  # final v -> DRAM

    nc.finalize()
    return nc


def kernel(x, W, bias):
    x = np.asarray(x, dtype=np.float32)
    W = np.asarray(W, dtype=np.float32)
    bias = np.asarray(bias, dtype=np.float32)

    from concourse.bass_utils import run_bass_kernel_spmd

    if "nc" not in _CACHE:
        _CACHE["nc"] = _build_bass()
    nc = _CACHE["nc"]

    in_maps = _pack_inputs(x, W, bias)
    res = run_bass_kernel_spmd(nc, in_maps, core_ids=list(range(NC)))
    _CACHE["last_results"] = res

    out = np.zeros((B, NO, K), dtype=np.float32)
    for cid in range(NC):
        v = res.results[cid]["out_v"]          # [32, 160] in (k,o) order
        out[cid * BL:(cid + 1) * BL] = (
            v.reshape(BL, K, NO).transpose(0, 2, 1))
    return out


if __name__ == "__main__":
    import reference
    inputs = reference.setup_inputs()
    inputs = {k: np.asarray(v) for k, v in inputs.items()}
    expected = np.asarray(reference.reference(**inputs))
    actual = kernel(**inputs)
    err = np.abs(actual - expected).max() / (np.abs(expected).max() + 1e-12)
    print("Relative error:", err)



# revision 109
# speedup vs baseline: 1.0049x; 1.0049x over previous
"""DigitCaps dynamic-routing kernel for 8 Trainium2 NeuronCores.

Strategy (batch-sharded, fully local per core, no collectives):
  B=256 -> 8 cores x 32 batch rows; u_hat resident in SBUF as bf16 in layout
      u_hat[p = 32*(i%4) + b, free = (i//4)*160 + k*10 + o]

  Creation: ONE full-array matmul per j-round (i = 4j+r): lhsT is a
  host-packed block-diagonal x image [32=(r,d), 128=(r,b)] so all 128 out
  partitions fill per instruction (4x fewer PE rows than per-quadrant
  tiling). PSUM drains to bf16 split DVE/ACT. s1 (uniform-c iteration 1)
  uses a dense (i,d)=128 contraction, sandwiched mid-creation so squash(v1)
  and the first cdiag build hide under the remaining drains.

  Agreement passes run on the PE as diagonal matmuls: cdiag[(ko)] =
  mask * v[p,ko] (built 2x-mode via pair duplication, per-o stripes,
  DVE+Pool) then out[128, 96-j] += diag(v_ko) @ u[:, (j, ko)], accumulating
  over k into rotating 1-bank PSUM pieces. Each piece's 16 accumulating
  matmuls are consecutive (interleaved start/stop groups in one bank
  corrupt sibling slots). Softmax state is eL = exp(logits) in SBUF bf16,
  o-major; iteration 3 uses the multiplicative update eL *= exp(a_delta)
  so no logits tensor is ever resident. Z = per-j reduce + reciprocal;
  c16 in one strided multiply per 96-j group.

  s-pass: t = c (.) u_hat on DVE (bf16 2x; Pool takes 2 of 9 blocks), then
  the PE folds sum_j sum_{i%4} via ones-block-diagonal accumulating
  matmuls into s PSUM [32,160]. The three 96-j groups are software-
  pipelined: PE agreement of group N+1 is emitted before the folds of
  group N so the PE never stalls on DVE.

  squash on [32,*] tiles (DVE + one ACT sqrt); v is replicated x4 via a
  onesT matmul and consumed straight from PSUM by the next cdiag build.
"""

import numpy as np

B, NI, DI, NO, K = 256, 1152, 8, 10, 16
NC = 8
BL = B // NC            # 32 batch rows per core
NJ = NI // 4            # 288 j-rounds (i = 4*j + r)
OK = NO * K             # 160, stored in (k, o) order: idx = k*10 + o
NQ = NI // 16           # 72 dense chunks (i = 16*q + t)
EPS = 1e-9

# creation PSUM chunking: 6 j-rounds per psum tile, 3 j per 512-col bank slot
JC = 6
# DVE block size (j per block) for routing passes
JB = 32

_CACHE = {}


def _pack_inputs(x, W, bias):
    """Host-side packing into per-core DMA images (all plain contiguous)."""
    import ml_dtypes
    bf16 = ml_dtypes.bfloat16

    # xblk (creation lhsT, block-diagonal over i_sub): [32, NJ*128]
    #   xblk[8*r' + d, j*128 + r*32 + b] = x[b0+b, 4*j+r, d] * (r'==r)
    # one [32,128] weight tile per j -> single full-array matmul per j.
    xr = x.reshape(B, NJ, 4, DI)                       # [b, j, r, d]
    xt_all = np.ascontiguousarray(
        xr.transpose(2, 3, 1, 0)).astype(bf16)         # [r, d, j, b_all]

    # dense xtD (s1 lhsT): [128, NQ*32]: xtD[16*?] rows = (t, d) = 128
    xd = x.reshape(B, NQ, 16, DI)                      # [b, q, t, d]
    xtD_all = np.ascontiguousarray(
        xd.transpose(2, 3, 1, 0)).astype(bf16)         # [t, d, q, b_all]

    # wp (creation rhs stream): dense rows [32, NJ*OK]
    #   wp[8*r + d, j*160 + k*10 + o] = W[4*j+r, o, k, d]
    wr = W.reshape(NJ, 4, NO, K, DI)                   # [j, r, o, k, d]
    wp = np.ascontiguousarray(
        wr.transpose(1, 4, 0, 3, 2).reshape(32, NJ * OK)).astype(bf16)

    # wd (s1 rhs stream, dense): [128, NQ*OK]
    #   wd[8*t + d, q*160 + k*10 + o] = W[16*q+t, o, k, d]
    wq = W.reshape(NQ, 16, NO, K, DI)                  # [q, t, o, k, d]
    wd = np.ascontiguousarray(
        wq.transpose(1, 4, 0, 3, 2).reshape(128, NQ * OK)).astype(bf16)

    # ones block-diag for folding 4 partition groups: [128, 32]
    ones_bd = np.zeros((128, BL), dtype=np.float32)
    for c in range(4):
        ones_bd[np.arange(BL) + 32 * c, np.arange(BL)] = 1.0
    ones_bd = ones_bd.astype(bf16)

    # onesT for partition replication via PE: [32, 128]
    onesT = np.zeros((BL, 128), dtype=np.float32)
    for c in range(4):
        onesT[np.arange(BL), np.arange(BL) + 32 * c] = 1.0
    onesT = onesT.astype(bf16)

    # bias replicated [32, 160] f32 in (k, o) order
    biasr = np.ascontiguousarray(
        np.broadcast_to(bias.T.reshape(1, OK), (BL, OK))).astype(np.float32)

    # identity mask for per-partition diag matmuls: [128, 128]
    mask = np.eye(128, dtype=np.float32).astype(bf16)
    xzero = np.zeros((32, 36 * 128), dtype=bf16)

    per_core = []
    for cid in range(NC):
        b0 = cid * BL
        xs = xt_all[:, :, :, b0:b0 + BL]               # [r, d, j, b]
        xblk = np.zeros((4, DI, NJ, 4, BL), dtype=bf16)
        for s in range(4):
            xblk[s, :, :, s, :] = xs[s]
        xblk = np.ascontiguousarray(xblk.reshape(32, NJ * 128))
        xtD = np.ascontiguousarray(
            xtD_all[:, :, :, b0:b0 + BL].reshape(128, NQ * BL))
        per_core.append({
            "xblk": xblk, "xtd": xtD, "wp": wp, "wd": wd,
            "ones_bd": ones_bd, "onesT": onesT, "biasr": biasr,
            "mask": mask,
        })
    return per_core


def _build_bass(debug=False, upto=99):
    import concourse.bass as bassm
    import concourse.bacc as bacc
    import concourse.mybir as mybir
    from concourse.tile import TileContext

    dt = mybir.dt
    ALU = mybir.AluOpType
    ACTF = mybir.ActivationFunctionType
    AX = mybir.AxisListType

    nc = bacc.Bacc()

    xblk_d = nc.dram_tensor("xblk", [32, NJ * 128], dt.bfloat16, kind="ExternalInput")
    xtD_d = nc.dram_tensor("xtd", [128, NQ * BL], dt.bfloat16, kind="ExternalInput")
    wp_d = nc.dram_tensor("wp", [32, NJ * OK], dt.bfloat16, kind="ExternalInput")
    wd_d = nc.dram_tensor("wd", [128, NQ * OK], dt.bfloat16, kind="ExternalInput")
    ones_d = nc.dram_tensor("ones_bd", [128, BL], dt.bfloat16, kind="ExternalInput")
    onesT_d = nc.dram_tensor("onesT", [BL, 128], dt.bfloat16, kind="ExternalInput")
    biasr_d = nc.dram_tensor("biasr", [BL, OK], dt.float32, kind="ExternalInput")
    mask_d = nc.dram_tensor("mask", [128, 128], dt.bfloat16, kind="ExternalInput")
    out_d = nc.dram_tensor("out_v", [BL, OK], dt.float32, kind="ExternalOutput")
    if debug:
        dbg_u = nc.dram_tensor("dbg_u", [128, NJ * OK], dt.bfloat16, kind="ExternalOutput")
        dbg_eL = nc.dram_tensor("dbg_eL", [128, NO * NJ], dt.bfloat16, kind="ExternalOutput")
        dbg_c = nc.dram_tensor("dbg_c", [128, NJ * NO], dt.bfloat16, kind="ExternalOutput")
        dbg_v1 = nc.dram_tensor("dbg_v1", [128, OK], dt.bfloat16, kind="ExternalOutput")

    with TileContext(nc) as tc:
        with (
            tc.tile_pool(name="const", bufs=1) as const,
            tc.tile_pool(name="big", bufs=1) as big,
            tc.tile_pool(name="small", bufs=1) as small,
            tc.tile_pool(name="pss", bufs=1, space="PSUM") as pss,
            tc.tile_pool(name="psA", bufs=2, space="PSUM") as psA,
        ):
            # ---- resident tiles ----
            u_hat = big.tile([128, NJ * OK], dt.bfloat16)     # 90KB/part
            cdiag = big.tile([128, OK * 128], dt.bfloat16)    # 40KB/part
            ones_sb = const.tile([128, BL], dt.bfloat16)
            onesT_sb = const.tile([BL, 128], dt.bfloat16)
            biasr_sb = const.tile([BL, OK], dt.float32)
            mask_sb = const.tile([128, 128], dt.bfloat16)
            vrep2 = const.tile([128, 2 * OK], dt.bfloat16)    # (ko, 2) pairs
            vrep_ps = [None]  # squash's replicate-PSUM, read by cdiag_build

            s1_ps = pss.tile([BL, OK], dt.float32, tag="sps")

            CD = cdiag.rearrange("p (t f) -> p t f", f=128)    # [p,160,128]
            UJ = u_hat.rearrange("p (j f) -> p j f", f=OK)     # [p,288,160]

            def cdiag_build():
                """cdiag[p,ko,f] = mask[p,f] * v[p,ko]; DVE+Pool.

                Reads v replicated straight from the squash's PSUM tile
                (vrep_ps[0]). The pair duplication vrep2[p,(ko,2)] makes
                the DVE ops' innermost dim a packed stride-1 pair -> 2x
                DVE mode."""
                v2v = vrep2.rearrange("p (t two) -> p t two", two=2)
                vin = vrep_ps[0].rearrange("p (t a) -> p t a", a=1)
                vi2, _ = bassm.broadcast_tensor_aps(vin, v2v)
                nc.vector.tensor_copy(v2v, vi2)
                mk4 = mask_sb.rearrange(
                    "p (a h two) -> p a h two", a=1, two=2)   # [p,1,64,2]
                vv5 = vrep2.rearrange(
                    "p (k o a two) -> p o k a two",
                    k=K, o=NO, a=1, two=2)                    # [p,10,16,1,2]
                CD5 = cdiag.rearrange(
                    "p (k o h two) -> p o k h two",
                    k=K, o=NO, h=64, two=2)                   # [p,10,16,64,2]
                # per-o stripes (pair index = k*10 + o: stride over k) so
                # the agreement pieces (o-major) can start as stripes land;
                # o=0..7 on DVE, o=8,9 on Pool
                for o in range(NO):
                    eng = nc.vector if o < 8 else nc.gpsimd
                    m2, v2 = bassm.broadcast_tensor_aps(mk4, vv5[:, o])
                    eng.tensor_tensor(CD5[:, o], m2, v2, ALU.mult)

            def squash_to_vrep(s_ps, store_out=False):
                """v = squash(s) from PSUM [32,160] (+bias);
                replicate to vrep [128,160] bf16 (or DMA out if final)."""
                s_sb = small.tile([BL, OK], dt.float32, tag="s_sb")
                nc.vector.scalar_tensor_tensor(
                    s_sb[:, :], s_ps[:, :], 0.1 if store_out is None else 1.0,
                    biasr_sb[:, :], ALU.mult, ALU.add)
                sq = small.tile([BL, OK], dt.float32, tag="sq")
                nc.vector.tensor_mul(sq[:, :], s_sb[:, :], s_sb[:, :])
                n2 = small.tile([BL, NO], dt.float32, tag="n2")
                nc.vector.tensor_reduce(
                    n2[:, :],
                    sq.rearrange("p (k o) -> p o k", o=NO),
                    AX.X, ALU.add)
                n2e = small.tile([BL, NO], dt.float32, tag="n2e")
                nc.vector.tensor_scalar_add(n2e[:, :], n2[:, :], EPS)
                sr = small.tile([BL, NO], dt.float32, tag="sr")
                nc.scalar.activation(sr[:, :], n2e[:, :], ACTF.Sqrt)
                den = small.tile([BL, NO], dt.float32, tag="den")
                nc.vector.scalar_tensor_tensor(
                    den[:, :], n2[:, :], 1.0, sr[:, :], ALU.add, ALU.mult)
                rec = small.tile([BL, NO], dt.float32, tag="rec")
                nc.vector.reciprocal(rec[:, :], den[:, :])
                g = small.tile([BL, NO], dt.float32, tag="g")
                nc.vector.tensor_mul(g[:, :], n2[:, :], rec[:, :])
                sv = s_sb.rearrange("p (k o) -> p k o", o=NO)
                gv = g.rearrange("p (a o) -> p a o", a=1)
                sv2, gv2 = bassm.broadcast_tensor_aps(sv, gv)
                if store_out:
                    v_sb = small.tile([BL, OK], dt.float32, tag="v_sb")
                    nc.vector.tensor_tensor(
                        v_sb.rearrange("p (k o) -> p k o", o=NO), sv2, gv2,
                        ALU.mult)
                    nc.sync.dma_start(out_d[:, :], v_sb[:, :])
                    return
                v16 = small.tile([BL, OK], dt.bfloat16, tag="v16")
                nc.vector.tensor_tensor(
                    v16.rearrange("p (k o) -> p k o", o=NO), sv2, gv2,
                    ALU.mult)
                vr_ps = pss.tile([128, OK], dt.float32, tag="vr_ps")
                nc.tensor.matmul(
                    vr_ps[:, :], onesT_sb[:, :], v16[:, :],
                    start=True, stop=True)
                vrep_ps[0] = vr_ps

            # =========== creation phase (scoped pools) ===========
            with (
                tc.tile_pool(name="xw", bufs=2) as xw,
                tc.tile_pool(name="crp", bufs=1) as crp,
                tc.tile_pool(name="ps", bufs=2, space="PSUM") as psp,
            ):
                xtD_sb = crp.tile([128, NQ * BL], dt.bfloat16)
                wd_sb = crp.tile([128, NQ * OK], dt.bfloat16)
                JD = 36

                def s1_and_v1():
                    # s1: dense (i,d)=128 contraction; v1 = squash(.1*s1+b)
                    for q in range(NQ):
                        nc.tensor.matmul(
                            s1_ps[:, :],
                            xtD_sb[:, q * BL:(q + 1) * BL],
                            wd_sb[:, q * OK:(q + 1) * OK],
                            start=(q == 0), stop=(q == NQ - 1),
                        )
                    squash_to_vrep(s1_ps, store_out=None)
                    cdiag_build()

                # u_hat: one [32x128] matmul per j (block-diag xb lhsT,
                # expanded on-chip from the dense xt stream)
                for ci, jd in enumerate(range(0, NJ, JD)):
                    xb = xw.tile([32, JD * 128], dt.bfloat16, tag="xb")
                    wp_ch = xw.tile([32, JD * OK], dt.bfloat16, tag="wp")
                    nc.sync.dma_start(
                        xb[:, :], xblk_d[:, jd * 128:(jd + JD) * 128])
                    nc.gpsimd.dma_start(
                        wp_ch[:, :], wp_d[:, jd * OK:(jd + JD) * OK])
                    if jd == JD:
                        # consts + s1 dense inputs: issue after the first
                        # stream chunks so they don't delay creation start
                        nc.sync.dma_start(ones_sb[:, :], ones_d[:, :])
                        nc.sync.dma_start(onesT_sb[:, :], onesT_d[:, :])
                        nc.sync.dma_start(biasr_sb[:, :], biasr_d[:, :])
                        nc.sync.dma_start(mask_sb[:, :], mask_d[:, :])
                        nc.sync.dma_start(xtD_sb[:, :], xtD_d[:, :])
                        nc.gpsimd.dma_start(wd_sb[:, :], wd_d[:, :])
                    if jd == 5 * JD:
                        # sandwich s1 + v1 + cdiag1 into the engine queues
                        # so they complete while late drains still run
                        s1_and_v1()
                    for jc in range(0, JD, JC):
                        cps = psp.tile([128, 1024], dt.float32, tag="cps")
                        for jj in range(JC):
                            off = (jj // 3) * 512 + (jj % 3) * OK
                            nc.tensor.matmul(
                                cps[:, off:off + OK],
                                xb[:, (jc + jj) * 128:(jc + jj + 1) * 128],
                                wp_ch[:, (jc + jj) * OK:(jc + jj + 1) * OK],
                                start=True, stop=True,
                            )
                        # drain 6 j (strided: 2 banks x 480 cols) -> bf16
                        src = cps.rearrange("p (a x) -> p a x", a=2)[:, :, 0:3 * OK]
                        dst = u_hat[:, (jd + jc) * OK:(jd + jc + JC) * OK].rearrange(
                            "p (a x) -> p a x", a=2)
                        di = (jd + jc) // JC
                        if di % 3 == 0 or (di >= 36 and di % 3 == 2):
                            nc.vector.tensor_copy(dst, src)
                        else:
                            nc.scalar.copy(dst, src)

            # =========== iteration helpers ===========
            # =========== routing phase ===========
            with tc.tile_pool(name="tmp", bufs=3) as tmpp:
                eL = tmpp.tile([128, NO * NJ], dt.bfloat16, bufs=1, tag="eL")
                z5 = tmpp.tile([128, 5 * NJ], dt.bfloat16, bufs=1, tag="z5")
                zrec = tmpp.tile([128, NJ], dt.bfloat16, bufs=1, tag="zrec")
                c16 = tmpp.tile([128, NJ * NO], dt.bfloat16, bufs=1, tag="c16")
                ELo = eL.rearrange("p (o j) -> p o j", j=NJ)   # [p,10,288]
                Z5 = z5.rearrange("p (o j) -> p o j", j=NJ)    # [p,5,288]
                Cj = c16.rearrange("p (j o) -> p j o", o=NO)   # [p,288,10]

                def routing_iter(update, build_diag):
                    """One fused routing iteration: agreement -> softmax ->
                    s-pass, pipelined across the three jp-groups of 96 j so
                    PE agreement of group N+1 overlaps DVE softmax/s-mult of
                    group N. Returns the s PSUM [32,160].

                    Agreement accumulates (over k) into rotating 1-bank PSUM
                    pieces [128, 5x96]; softmax state is eL (SBUF bf16).
                    update=False: eL = exp(a); update=True (later iters):
                    eL *= exp(a_delta)  -- multiplicative softmax update, no
                    resident logits needed.

                    Pool computes the slow s-mult t-blocks 0,1 (their c16
                    group is ready first); their folds close the chain so
                    PE never waits on Pool."""
                    if build_diag:
                        cdiag_build()
                    s_ps = pss.tile([BL, OK], dt.float32, tag="sps")
                    # fold order: pool blocks 0,1 last; first fold = block 2
                    fold_sched = {0: [2], 1: [3, 0, 4, 5], 2: [6, 1, 7, 8]}
                    t_tiles = {}
                    started = [False]

                    def emit_agr(jp):
                        # k outer: the first matmuls need only the first
                        # cdiag range, so agreement starts ~1us into the
                        # cdiag build instead of after it
                        aps = []
                        for oh in range(2):
                            ap = psA.tile([128, 480], dt.float32, tag="ap",
                                          name=f"ap{jp}{oh}")
                            aps.append(ap)
                        # o outer, k inner: each piece's 16 accumulating
                        # matmuls are CONSECUTIVE -- interleaved start/stop
                        # groups within one PSUM bank corrupt earlier slots
                        for o in range(NO):
                            for k in range(K):
                                ko = k * NO + o
                                slot = o % 5
                                nc.tensor.matmul(
                                    aps[o // 5][:, slot * 96:(slot + 1) * 96],
                                    CD[:, ko, :],
                                    UJ[:, jp * 96:(jp + 1) * 96, ko],
                                    start=(k == 0),
                                    stop=(k == K - 1),
                                )
                        return aps

                    def emit_softmax(jp, aps):
                        js = slice(jp * 96, (jp + 1) * 96)
                        for oh in range(2):
                            src = aps[oh].rearrange("p (s j) -> p s j", j=96)
                            dstv = ELo[:, oh * 5:(oh + 1) * 5, js]
                            if not update:
                                nc.scalar.activation(dstv, src, ACTF.Exp)
                            else:
                                ex = tmpp.tile([128, 480], dt.bfloat16,
                                               tag="ex", name="ex", bufs=2)
                                exv = ex.rearrange("p (s j) -> p s j", j=96)
                                nc.scalar.activation(exv, src, ACTF.Exp)
                                nc.vector.tensor_tensor(
                                    dstv, dstv, exv, ALU.mult)
                        eJO = eL.rearrange("p (o j) -> p j o", o=NO)
                        with nc.allow_low_precision(
                                reason="Z/c in bf16; error averages over i"):
                            nc.vector.tensor_reduce(
                                z5[:, js], eJO[:, js, :], AX.X, ALU.add)
                            nc.vector.reciprocal(zrec[:, js], z5[:, js])
                        zb = zrec.rearrange("p (j a) -> p j a", a=1)
                        e2, z2 = bassm.broadcast_tensor_aps(
                            eJO[:, js, :], zb[:, js, :])
                        nc.vector.tensor_tensor(
                            Cj[:, js, :], e2, z2, ALU.mult)

                    def emit_smult(jp):
                        # blocks 0,1 on Pool; the rest on DVE
                        for bi in (3 * jp, 3 * jp + 1, 3 * jp + 2):
                            jb = bi * JB
                            pool_blk = bi <= 1
                            tag = "tp" if pool_blk else "t"
                            t = tmpp.tile([128, JB * OK], dt.bfloat16,
                                          tag=tag, name=f"t_{tag}",
                                          bufs=2 if pool_blk else 3)
                            t_tiles[bi] = t
                            tv = t.rearrange("p (j k o) -> p j k o",
                                             j=JB, k=K)
                            uv = u_hat[:, jb * OK:(jb + JB) * OK].rearrange(
                                "p (j k o) -> p j k o", j=JB, k=K)
                            cv = Cj[:, jb:jb + JB, :].rearrange(
                                "p j (a o) -> p j a o", a=1)
                            uv2, cv2 = bassm.broadcast_tensor_aps(uv, cv)
                            eng = nc.gpsimd if pool_blk else nc.vector
                            eng.tensor_tensor(tv, uv2, cv2, ALU.mult)

                    def emit_folds(jp):
                        for bi in fold_sched[jp]:
                            t = t_tiles[bi]
                            for jj in range(JB):
                                nc.tensor.matmul(
                                    s_ps[:, :], ones_sb[:, :],
                                    t[:, jj * OK:(jj + 1) * OK],
                                    start=not started[0],
                                    stop=(bi == 8 and jj == JB - 1))
                                started[0] = True

                    # software-pipelined: folds of group N trail the
                    # agreement matmuls of group N+1 on the PE queue
                    ap0 = emit_agr(0)
                    emit_softmax(0, ap0)
                    emit_smult(0)
                    ap1 = emit_agr(1)
                    emit_folds(0)
                    emit_softmax(1, ap1)
                    emit_smult(1)
                    ap2 = emit_agr(2)
                    emit_folds(1)
                    emit_softmax(2, ap2)
                    emit_smult(2)
                    emit_folds(2)
                    return s_ps

                # =========== routing ===========
                # (iter 1 = s1+squash1+cdiag1 was sandwiched mid-creation)
                if debug:
                    nc.sync.dma_start(dbg_u[:, :], u_hat[:, :])
                    nc.sync.dma_start(
                        dbg_v1[:, :],
                        vrep2.rearrange("p (t two) -> p t two", two=2)[:, :, 0])
                # iter 2 (cdiag1 already built mid-creation)
                s2 = routing_iter(update=False, build_diag=False)
                if debug:
                    nc.sync.dma_start(dbg_eL[:, :], eL[:, :])
                    nc.sync.dma_start(dbg_c[:, :], c16[:, :])
                squash_to_vrep(s2)                  # v2 -> vrep
                # iter 3: eL *= exp(a2)
                s3 = routing_iter(update=True, build_diag=True)
                squash_to_vrep(s3, store_out=True)  # final v -> DRAM

    nc.finalize()
    return nc


def kernel(x, W, bias):
    x = np.asarray(x, dtype=np.float32)
    W = np.asarray(W, dtype=np.float32)
    bias = np.asarray(bias, dtype=np.float32)

    from concourse.bass_utils import run_bass_kernel_spmd

    if "nc" not in _CACHE:
        _CACHE["nc"] = _build_bass()
    nc = _CACHE["nc"]

    in_maps = _pack_inputs(x, W, bias)
    res = run_bass_kernel_spmd(nc, in_maps, core_ids=list(range(NC)))
    _CACHE["last_results"] = res

    out = np.zeros((B, NO, K), dtype=np.float32)
    for cid in range(NC):
        v = res.results[cid]["out_v"]          # [32, 160] in (k,o) order
        out[cid * BL:(cid + 1) * BL] = (
            v.reshape(BL, K, NO).transpose(0, 2, 1))
    return out


if __name__ == "__main__":
    import reference
    inputs = reference.setup_inputs()
    inputs = {k: np.asarray(v) for k, v in inputs.items()}
    expected = np.asarray(reference.reference(**inputs))
    actual = kernel(**inputs)
    err = np.abs(actual - expected).max() / (np.abs(expected).max() + 1e-12)
    print("Relative error:", err)



# revision 114
# speedup vs baseline: 1.0077x; 1.0028x over previous
"""DigitCaps dynamic-routing kernel for 8 Trainium2 NeuronCores.

Strategy (batch-sharded, fully local per core, no collectives):
  B=256 -> 8 cores x 32 batch rows; u_hat resident in SBUF as bf16 in layout
      u_hat[p = 32*(i%4) + b, free = (i//4)*160 + k*10 + o]

  Creation: ONE full-array matmul per j-round (i = 4j+r): lhsT is a
  host-packed block-diagonal x image [32=(r,d), 128=(r,b)] so all 128 out
  partitions fill per instruction (4x fewer PE rows than per-quadrant
  tiling). PSUM drains to bf16 split DVE/ACT. s1 (uniform-c iteration 1)
  uses a dense (i,d)=128 contraction, sandwiched mid-creation so squash(v1)
  and the first cdiag build hide under the remaining drains.

  Agreement passes run on the PE as diagonal matmuls: cdiag[(ko)] =
  mask * v[p,ko] (built 2x-mode via pair duplication, per-o stripes,
  DVE+Pool) then out[128, 96-j] += diag(v_ko) @ u[:, (j, ko)], accumulating
  over k into rotating 1-bank PSUM pieces. Each piece's 16 accumulating
  matmuls are consecutive (interleaved start/stop groups in one bank
  corrupt sibling slots). Softmax state is eL = exp(logits) in SBUF bf16,
  o-major; iteration 3 uses the multiplicative update eL *= exp(a_delta)
  so no logits tensor is ever resident. Z = per-j reduce + reciprocal;
  c16 in one strided multiply per 96-j group.

  s-pass: t = c (.) u_hat on DVE (bf16 2x; Pool takes 2 of 9 blocks), then
  the PE folds sum_j sum_{i%4} via ones-block-diagonal accumulating
  matmuls into s PSUM [32,160]. The three 96-j groups are software-
  pipelined: PE agreement of group N+1 is emitted before the folds of
  group N so the PE never stalls on DVE.

  squash on [32,*] tiles (DVE + one ACT sqrt); v is replicated x4 via a
  onesT matmul and consumed straight from PSUM by the next cdiag build.
"""

import numpy as np

B, NI, DI, NO, K = 256, 1152, 8, 10, 16
NC = 8
BL = B // NC            # 32 batch rows per core
NJ = NI // 4            # 288 j-rounds (i = 4*j + r)
OK = NO * K             # 160, stored in (k, o) order: idx = k*10 + o
NQ = NI // 16           # 72 dense chunks (i = 16*q + t)
EPS = 1e-9

# creation PSUM chunking: 6 j-rounds per psum tile, 3 j per 512-col bank slot
JC = 6
# DVE block size (j per block) for routing passes
JB = 32

_CACHE = {}


def _pack_inputs(x, W, bias):
    """Host-side packing into per-core DMA images (all plain contiguous)."""
    import ml_dtypes
    bf16 = ml_dtypes.bfloat16

    # xblk (creation lhsT, block-diagonal over i_sub): [32, NJ*128]
    #   xblk[8*r' + d, j*128 + r*32 + b] = x[b0+b, 4*j+r, d] * (r'==r)
    # one [32,128] weight tile per j -> single full-array matmul per j.
    xr = x.reshape(B, NJ, 4, DI)                       # [b, j, r, d]
    xt_all = np.ascontiguousarray(
        xr.transpose(2, 3, 1, 0)).astype(bf16)         # [r, d, j, b_all]

    # dense xtD (s1 lhsT): [128, NQ*32]: xtD[16*?] rows = (t, d) = 128
    xd = x.reshape(B, NQ, 16, DI)                      # [b, q, t, d]
    xtD_all = np.ascontiguousarray(
        xd.transpose(2, 3, 1, 0)).astype(bf16)         # [t, d, q, b_all]

    # wp (creation rhs stream): dense rows [32, NJ*OK]
    #   wp[8*r + d, j*160 + k*10 + o] = W[4*j+r, o, k, d]
    wr = W.reshape(NJ, 4, NO, K, DI)                   # [j, r, o, k, d]
    wp = np.ascontiguousarray(
        wr.transpose(1, 4, 0, 3, 2).reshape(32, NJ * OK)).astype(bf16)

    # wd (s1 rhs stream, dense): [128, NQ*OK]
    #   wd[8*t + d, q*160 + k*10 + o] = W[16*q+t, o, k, d]
    wq = W.reshape(NQ, 16, NO, K, DI)                  # [q, t, o, k, d]
    wd = np.ascontiguousarray(
        wq.transpose(1, 4, 0, 3, 2).reshape(128, NQ * OK)).astype(bf16)

    # ones block-diag for folding 4 partition groups: [128, 32]
    ones_bd = np.zeros((128, BL), dtype=np.float32)
    for c in range(4):
        ones_bd[np.arange(BL) + 32 * c, np.arange(BL)] = 1.0
    ones_bd = ones_bd.astype(bf16)

    # onesT for partition replication via PE: [32, 128]
    onesT = np.zeros((BL, 128), dtype=np.float32)
    for c in range(4):
        onesT[np.arange(BL), np.arange(BL) + 32 * c] = 1.0
    onesT = onesT.astype(bf16)

    # bias replicated [32, 160] f32 in (k, o) order
    biasr = np.ascontiguousarray(
        np.broadcast_to(bias.T.reshape(1, OK), (BL, OK))).astype(np.float32)

    # identity mask for per-partition diag matmuls: [128, 128]
    mask = np.eye(128, dtype=np.float32).astype(bf16)
    xzero = np.zeros((32, 36 * 128), dtype=bf16)

    per_core = []
    for cid in range(NC):
        b0 = cid * BL
        xs = xt_all[:, :, :, b0:b0 + BL]               # [r, d, j, b]
        xblk = np.zeros((4, DI, NJ, 4, BL), dtype=bf16)
        for s in range(4):
            xblk[s, :, :, s, :] = xs[s]
        xblk = np.ascontiguousarray(xblk.reshape(32, NJ * 128))
        xtD = np.ascontiguousarray(
            xtD_all[:, :, :, b0:b0 + BL].reshape(128, NQ * BL))
        per_core.append({
            "xblk": xblk, "xtd": xtD, "wp": wp, "wd": wd,
            "ones_bd": ones_bd, "onesT": onesT, "biasr": biasr,
            "mask": mask,
        })
    return per_core


def _build_bass(debug=False, upto=99):
    import concourse.bass as bassm
    import concourse.bacc as bacc
    import concourse.mybir as mybir
    from concourse.tile import TileContext

    dt = mybir.dt
    ALU = mybir.AluOpType
    ACTF = mybir.ActivationFunctionType
    AX = mybir.AxisListType

    nc = bacc.Bacc()

    xblk_d = nc.dram_tensor("xblk", [32, NJ * 128], dt.bfloat16, kind="ExternalInput")
    xtD_d = nc.dram_tensor("xtd", [128, NQ * BL], dt.bfloat16, kind="ExternalInput")
    wp_d = nc.dram_tensor("wp", [32, NJ * OK], dt.bfloat16, kind="ExternalInput")
    wd_d = nc.dram_tensor("wd", [128, NQ * OK], dt.bfloat16, kind="ExternalInput")
    ones_d = nc.dram_tensor("ones_bd", [128, BL], dt.bfloat16, kind="ExternalInput")
    onesT_d = nc.dram_tensor("onesT", [BL, 128], dt.bfloat16, kind="ExternalInput")
    biasr_d = nc.dram_tensor("biasr", [BL, OK], dt.float32, kind="ExternalInput")
    mask_d = nc.dram_tensor("mask", [128, 128], dt.bfloat16, kind="ExternalInput")
    out_d = nc.dram_tensor("out_v", [BL, OK], dt.float32, kind="ExternalOutput")
    if debug:
        dbg_u = nc.dram_tensor("dbg_u", [128, NJ * OK], dt.bfloat16, kind="ExternalOutput")
        dbg_eL = nc.dram_tensor("dbg_eL", [128, NO * NJ], dt.bfloat16, kind="ExternalOutput")
        dbg_c = nc.dram_tensor("dbg_c", [128, NJ * NO], dt.bfloat16, kind="ExternalOutput")
        dbg_v1 = nc.dram_tensor("dbg_v1", [128, OK], dt.bfloat16, kind="ExternalOutput")

    with TileContext(nc) as tc:
        with (
            tc.tile_pool(name="const", bufs=1) as const,
            tc.tile_pool(name="big", bufs=1) as big,
            tc.tile_pool(name="small", bufs=1) as small,
            tc.tile_pool(name="pss", bufs=1, space="PSUM") as pss,
            tc.tile_pool(name="psA", bufs=2, space="PSUM") as psA,
        ):
            # ---- resident tiles ----
            u_hat = big.tile([128, NJ * OK], dt.bfloat16)     # 90KB/part
            cdiag = big.tile([128, OK * 128], dt.bfloat16)    # 40KB/part
            ones_sb = const.tile([128, BL], dt.bfloat16)
            onesT_sb = const.tile([BL, 128], dt.bfloat16)
            biasr_sb = const.tile([BL, OK], dt.float32)
            mask_sb = const.tile([128, 128], dt.bfloat16)
            vrep2 = const.tile([128, 2 * OK], dt.bfloat16)    # (ko, 2) pairs
            eps_sb = const.tile([BL, 1], dt.float32)
            nc.gpsimd.memset(eps_sb[:, :], EPS)
            vrep_ps = [None]  # squash's replicate-PSUM, read by cdiag_build

            s1_ps = pss.tile([BL, OK], dt.float32, tag="sps")

            CD = cdiag.rearrange("p (t f) -> p t f", f=128)    # [p,160,128]
            UJ = u_hat.rearrange("p (j f) -> p j f", f=OK)     # [p,288,160]

            def cdiag_build():
                """cdiag[p,ko,f] = mask[p,f] * v[p,ko]; DVE+Pool.

                Reads v replicated straight from the squash's PSUM tile
                (vrep_ps[0]). The pair duplication vrep2[p,(ko,2)] makes
                the DVE ops' innermost dim a packed stride-1 pair -> 2x
                DVE mode."""
                v2v = vrep2.rearrange("p (t two) -> p t two", two=2)
                vin = vrep_ps[0].rearrange("p (t a) -> p t a", a=1)
                vi2, _ = bassm.broadcast_tensor_aps(vin, v2v)
                nc.vector.tensor_copy(v2v, vi2)
                mk4 = mask_sb.rearrange(
                    "p (a h two) -> p a h two", a=1, two=2)   # [p,1,64,2]
                vv5 = vrep2.rearrange(
                    "p (k o a two) -> p o k a two",
                    k=K, o=NO, a=1, two=2)                    # [p,10,16,1,2]
                CD5 = cdiag.rearrange(
                    "p (k o h two) -> p o k h two",
                    k=K, o=NO, h=64, two=2)                   # [p,10,16,64,2]
                # per-o stripes (pair index = k*10 + o: stride over k) so
                # the agreement pieces (o-major) can start as stripes land;
                # o=0..7 on DVE, o=8,9 on Pool
                for o in range(NO):
                    eng = nc.vector if o < 8 else nc.gpsimd
                    m2, v2 = bassm.broadcast_tensor_aps(mk4, vv5[:, o])
                    eng.tensor_tensor(CD5[:, o], m2, v2, ALU.mult)

            def squash_to_vrep(s_ps, store_out=False):
                """v = squash(s) from PSUM [32,160] (+bias);
                replicate to vrep [128,160] bf16 (or DMA out if final)."""
                s_sb = small.tile([BL, OK], dt.float32, tag="s_sb")
                nc.vector.scalar_tensor_tensor(
                    s_sb[:, :], s_ps[:, :], 0.1 if store_out is None else 1.0,
                    biasr_sb[:, :], ALU.mult, ALU.add)
                sq = small.tile([BL, OK], dt.float32, tag="sq")
                nc.vector.tensor_mul(sq[:, :], s_sb[:, :], s_sb[:, :])
                n2 = small.tile([BL, NO], dt.float32, tag="n2")
                nc.vector.tensor_reduce(
                    n2[:, :],
                    sq.rearrange("p (k o) -> p o k", o=NO),
                    AX.X, ALU.add)
                sr = small.tile([BL, NO], dt.float32, tag="sr")
                nc.scalar.activation(sr[:, :], n2[:, :], ACTF.Sqrt,
                                     bias=eps_sb[:, :], scale=1.0)
                den = small.tile([BL, NO], dt.float32, tag="den")
                nc.vector.scalar_tensor_tensor(
                    den[:, :], n2[:, :], 1.0, sr[:, :], ALU.add, ALU.mult)
                rec = small.tile([BL, NO], dt.float32, tag="rec")
                nc.vector.reciprocal(rec[:, :], den[:, :])
                g = small.tile([BL, NO], dt.float32, tag="g")
                nc.vector.tensor_mul(g[:, :], n2[:, :], rec[:, :])
                sv = s_sb.rearrange("p (k o) -> p k o", o=NO)
                gv = g.rearrange("p (a o) -> p a o", a=1)
                sv2, gv2 = bassm.broadcast_tensor_aps(sv, gv)
                if store_out:
                    v_sb = small.tile([BL, OK], dt.float32, tag="v_sb")
                    nc.vector.tensor_tensor(
                        v_sb.rearrange("p (k o) -> p k o", o=NO), sv2, gv2,
                        ALU.mult)
                    nc.sync.dma_start(out_d[:, :], v_sb[:, :])
                    return
                v16 = small.tile([BL, OK], dt.bfloat16, tag="v16")
                nc.vector.tensor_tensor(
                    v16.rearrange("p (k o) -> p k o", o=NO), sv2, gv2,
                    ALU.mult)
                vr_ps = pss.tile([128, OK], dt.float32, tag="vr_ps")
                nc.tensor.matmul(
                    vr_ps[:, :], onesT_sb[:, :], v16[:, :],
                    start=True, stop=True)
                vrep_ps[0] = vr_ps

            # =========== creation phase (scoped pools) ===========
            with (
                tc.tile_pool(name="xw", bufs=2) as xw,
                tc.tile_pool(name="crp", bufs=1) as crp,
                tc.tile_pool(name="ps", bufs=2, space="PSUM") as psp,
            ):
                xtD_sb = crp.tile([128, NQ * BL], dt.bfloat16)
                wd_sb = crp.tile([128, NQ * OK], dt.bfloat16)
                JD = 36

                def s1_and_v1():
                    # s1: dense (i,d)=128 contraction; v1 = squash(.1*s1+b)
                    for q in range(NQ):
                        nc.tensor.matmul(
                            s1_ps[:, :],
                            xtD_sb[:, q * BL:(q + 1) * BL],
                            wd_sb[:, q * OK:(q + 1) * OK],
                            start=(q == 0), stop=(q == NQ - 1),
                        )
                    squash_to_vrep(s1_ps, store_out=None)
                    cdiag_build()

                # u_hat: one [32x128] matmul per j (block-diag xb lhsT,
                # expanded on-chip from the dense xt stream)
                for ci, jd in enumerate(range(0, NJ, JD)):
                    xb = xw.tile([32, JD * 128], dt.bfloat16, tag="xb")
                    wp_ch = xw.tile([32, JD * OK], dt.bfloat16, tag="wp")
                    nc.sync.dma_start(
                        xb[:, :], xblk_d[:, jd * 128:(jd + JD) * 128])
                    nc.gpsimd.dma_start(
                        wp_ch[:, :], wp_d[:, jd * OK:(jd + JD) * OK])
                    if jd == JD:
                        # consts + s1 dense inputs: issue after the first
                        # stream chunks so they don't delay creation start
                        nc.sync.dma_start(ones_sb[:, :], ones_d[:, :])
                        nc.sync.dma_start(onesT_sb[:, :], onesT_d[:, :])
                        nc.sync.dma_start(biasr_sb[:, :], biasr_d[:, :])
                        nc.sync.dma_start(mask_sb[:, :], mask_d[:, :])
                        nc.sync.dma_start(xtD_sb[:, :], xtD_d[:, :])
                        nc.gpsimd.dma_start(wd_sb[:, :], wd_d[:, :])
                    if jd == 5 * JD:
                        # sandwich s1 + v1 + cdiag1 into the engine queues
                        # so they complete while late drains still run
                        s1_and_v1()
                    for jc in range(0, JD, JC):
                        cps = psp.tile([128, 1024], dt.float32, tag="cps")
                        for jj in range(JC):
                            off = (jj // 3) * 512 + (jj % 3) * OK
                            nc.tensor.matmul(
                                cps[:, off:off + OK],
                                xb[:, (jc + jj) * 128:(jc + jj + 1) * 128],
                                wp_ch[:, (jc + jj) * OK:(jc + jj + 1) * OK],
                                start=True, stop=True,
                            )
                        # drain 6 j (strided: 2 banks x 480 cols) -> bf16
                        src = cps.rearrange("p (a x) -> p a x", a=2)[:, :, 0:3 * OK]
                        dst = u_hat[:, (jd + jc) * OK:(jd + jc + JC) * OK].rearrange(
                            "p (a x) -> p a x", a=2)
                        di = (jd + jc) // JC
                        if di % 3 == 0 or (di >= 36 and di % 3 == 2):
                            nc.vector.tensor_copy(dst, src)
                        else:
                            nc.scalar.copy(dst, src)

            # =========== iteration helpers ===========
            # =========== routing phase ===========
            with tc.tile_pool(name="tmp", bufs=3) as tmpp:
                eL = tmpp.tile([128, NO * NJ], dt.bfloat16, bufs=1, tag="eL")
                z5 = tmpp.tile([128, 5 * NJ], dt.bfloat16, bufs=1, tag="z5")
                zrec = tmpp.tile([128, NJ], dt.bfloat16, bufs=1, tag="zrec")
                c16 = tmpp.tile([128, NJ * NO], dt.bfloat16, bufs=1, tag="c16")
                ELo = eL.rearrange("p (o j) -> p o j", j=NJ)   # [p,10,288]
                Z5 = z5.rearrange("p (o j) -> p o j", j=NJ)    # [p,5,288]
                Cj = c16.rearrange("p (j o) -> p j o", o=NO)   # [p,288,10]

                def routing_iter(update, build_diag):
                    """One fused routing iteration: agreement -> softmax ->
                    s-pass, pipelined across the three jp-groups of 96 j so
                    PE agreement of group N+1 overlaps DVE softmax/s-mult of
                    group N. Returns the s PSUM [32,160].

                    Agreement accumulates (over k) into rotating 1-bank PSUM
                    pieces [128, 5x96]; softmax state is eL (SBUF bf16).
                    update=False: eL = exp(a); update=True (later iters):
                    eL *= exp(a_delta)  -- multiplicative softmax update, no
                    resident logits needed.

                    Pool computes the slow s-mult t-blocks 0,1 (their c16
                    group is ready first); their folds close the chain so
                    PE never waits on Pool."""
                    if build_diag:
                        cdiag_build()
                    s_ps = pss.tile([BL, OK], dt.float32, tag="sps")
                    # fold order: pool blocks 0,1 last; first fold = block 2
                    fold_sched = {0: [2], 1: [3, 0, 4, 5], 2: [6, 1, 7, 8]}
                    t_tiles = {}
                    started = [False]

                    def emit_agr(jp):
                        # k outer: the first matmuls need only the first
                        # cdiag range, so agreement starts ~1us into the
                        # cdiag build instead of after it
                        aps = []
                        for oh in range(2):
                            ap = psA.tile([128, 480], dt.float32, tag="ap",
                                          name=f"ap{jp}{oh}")
                            aps.append(ap)
                        # o outer, k inner: each piece's 16 accumulating
                        # matmuls are CONSECUTIVE -- interleaved start/stop
                        # groups within one PSUM bank corrupt earlier slots
                        for o in range(NO):
                            for k in range(K):
                                ko = k * NO + o
                                slot = o % 5
                                nc.tensor.matmul(
                                    aps[o // 5][:, slot * 96:(slot + 1) * 96],
                                    CD[:, ko, :],
                                    UJ[:, jp * 96:(jp + 1) * 96, ko],
                                    start=(k == 0),
                                    stop=(k == K - 1),
                                )
                        return aps

                    def emit_softmax(jp, aps):
                        js = slice(jp * 96, (jp + 1) * 96)
                        for oh in range(2):
                            src = aps[oh].rearrange("p (s j) -> p s j", j=96)
                            dstv = ELo[:, oh * 5:(oh + 1) * 5, js]
                            if not update:
                                nc.scalar.activation(dstv, src, ACTF.Exp)
                            else:
                                ex = tmpp.tile([128, 480], dt.bfloat16,
                                               tag="ex", name="ex", bufs=2)
                                exv = ex.rearrange("p (s j) -> p s j", j=96)
                                nc.scalar.activation(exv, src, ACTF.Exp)
                                nc.vector.tensor_tensor(
                                    dstv, dstv, exv, ALU.mult)
                        eJO = eL.rearrange("p (o j) -> p j o", o=NO)
                        with nc.allow_low_precision(
                                reason="Z/c in bf16; error averages over i"):
                            nc.vector.tensor_reduce(
                                z5[:, js], eJO[:, js, :], AX.X, ALU.add)
                            nc.vector.reciprocal(zrec[:, js], z5[:, js])
                        zb = zrec.rearrange("p (j a) -> p j a", a=1)
                        e2, z2 = bassm.broadcast_tensor_aps(
                            eJO[:, js, :], zb[:, js, :])
                        nc.vector.tensor_tensor(
                            Cj[:, js, :], e2, z2, ALU.mult)

                    def emit_smult(jp):
                        # blocks 0,1 on Pool; the rest on DVE
                        for bi in (3 * jp, 3 * jp + 1, 3 * jp + 2):
                            jb = bi * JB
                            pool_blk = bi <= 1
                            tag = "tp" if pool_blk else "t"
                            t = tmpp.tile([128, JB * OK], dt.bfloat16,
                                          tag=tag, name=f"t_{tag}",
                                          bufs=2 if pool_blk else 3)
                            t_tiles[bi] = t
                            tv = t.rearrange("p (j k o) -> p j k o",
                                             j=JB, k=K)
                            uv = u_hat[:, jb * OK:(jb + JB) * OK].rearrange(
                                "p (j k o) -> p j k o", j=JB, k=K)
                            cv = Cj[:, jb:jb + JB, :].rearrange(
                                "p j (a o) -> p j a o", a=1)
                            uv2, cv2 = bassm.broadcast_tensor_aps(uv, cv)
                            eng = nc.gpsimd if pool_blk else nc.vector
                            eng.tensor_tensor(tv, uv2, cv2, ALU.mult)

                    def emit_folds(jp):
                        for bi in fold_sched[jp]:
                            t = t_tiles[bi]
                            for jj in range(JB):
                                nc.tensor.matmul(
                                    s_ps[:, :], ones_sb[:, :],
                                    t[:, jj * OK:(jj + 1) * OK],
                                    start=not started[0],
                                    stop=(bi == 8 and jj == JB - 1))
                                started[0] = True

                    # software-pipelined: folds of group N trail the
                    # agreement matmuls of group N+1 on the PE queue
                    ap0 = emit_agr(0)
                    emit_softmax(0, ap0)
                    emit_smult(0)
                    ap1 = emit_agr(1)
                    emit_folds(0)
                    emit_softmax(1, ap1)
                    emit_smult(1)
                    ap2 = emit_agr(2)
                    emit_folds(1)
                    emit_softmax(2, ap2)
                    emit_smult(2)
                    emit_folds(2)
                    return s_ps

                # =========== routing ===========
                # (iter 1 = s1+squash1+cdiag1 was sandwiched mid-creation)
                if debug:
                    nc.sync.dma_start(dbg_u[:, :], u_hat[:, :])
                    nc.sync.dma_start(
                        dbg_v1[:, :],
                        vrep2.rearrange("p (t two) -> p t two", two=2)[:, :, 0])
                # iter 2 (cdiag1 already built mid-creation)
                s2 = routing_iter(update=False, build_diag=False)
                if debug:
                    nc.sync.dma_start(dbg_eL[:, :], eL[:, :])
                    nc.sync.dma_start(dbg_c[:, :], c16[:, :])
                squash_to_vrep(s2)                  # v2 -> vrep
                # iter 3: eL *= exp(a2)
                s3 = routing_iter(update=True, build_diag=True)
                squash_to_vrep(s3, store_out=True)  # final v -> DRAM

    nc.finalize()
    return nc


def kernel(x, W, bias):
    x = np.asarray(x, dtype=np.float32)
    W = np.asarray(W, dtype=np.float32)
    bias = np.asarray(bias, dtype=np.float32)

    from concourse.bass_utils import run_bass_kernel_spmd

    if "nc" not in _CACHE:
        _CACHE["nc"] = _build_bass()
    nc = _CACHE["nc"]

    in_maps = _pack_inputs(x, W, bias)
    res = run_bass_kernel_spmd(nc, in_maps, core_ids=list(range(NC)))
    _CACHE["last_results"] = res

    out = np.zeros((B, NO, K), dtype=np.float32)
    for cid in range(NC):
        v = res.results[cid]["out_v"]          # [32, 160] in (k,o) order
        out[cid * BL:(cid + 1) * BL] = (
            v.reshape(BL, K, NO).transpose(0, 2, 1))
    return out


if __name__ == "__main__":
    import reference
    inputs = reference.setup_inputs()
    inputs = {k: np.asarray(v) for k, v in inputs.items()}
    expected = np.asarray(reference.reference(**inputs))
    actual = kernel(**inputs)
    err = np.abs(actual - expected).max() / (np.abs(expected).max() + 1e-12)
    print("Relative error:", err)

